# revision 44
# baseline (speedup 1.0000x reference)
"""Trainium2 Bass kernel for suffix-softmax attention visualization.

Computes, for hidden_states [S, B, H], W [H, 1], b [1]:
    s[t, b]   = sum_h hidden_states[t, b, h] * W[h, 0] + b[0]
    out[t, b] = exp(s[t, b]) / sum_{t' >= t} exp(s[t', b])     (suffix softmax)
returned as [S, B, 1] f32.

The softmax ratio is shift-invariant, so the scalar bias b cancels exactly
and is not needed on device. Scores are N(0,1)-scaled by construction, so
exp() needs no max-subtraction.

Sharding: data-parallel over the batch axis — 8 NeuronCores, 8 batch
columns each; 64 MB of f32 per core, a pure HBM stream.

Architecture (v4, superblock). Measured context:
  - The input must land in SBUF as fp16 via the SWDGE cast-DMA: raw-f32
    landings double SBUF port traffic past the ~7 KB/us/partition wall
    (engines degrade 2.5-4x, measured 310-547 us), and only SWDGE can
    cast. HWDGE is immune to the engine-15 problem below but cannot
    cast, so it is not usable for the input.
  - Under SWDGE, SDMA engine 15 SOMETIMES (run-to-run lottery, roughly
    half of runs; device/co-tenant state) suffers periodic stall
    clusters making it ~20% slower than the other 15 engines; engines
    own fixed partition sets, so the whole stream ends up paced by
    engine 15. Separately, some runs throttle ALL engines ~17%.
  - This kernel streams in 512-seq-row SUPERBLOCKS: partition p holds
    the four consecutive rows t = 512k + 4p + i. Per partition that is
    one contiguous 64 KB f32 read -> 32 KB fp16 write descriptor — 4x
    fewer descriptors and semaphore packets than row-per-partition
    blocks and near-asymptotic per-packet efficiency (measured 163 us
    per-engine stream busy vs 169 us for the row-per-partition
    baseline; clean-run total 210.6 us vs 231.9 us).
  - Steady state issues ONE cast-DMA call per supertile: more
    outstanding calls overflow the SWDGE descriptor ring and stall the
    stream (measured 2-6 us gaps per supertile with 2 calls; same for
    look > 2). The first/last two supertiles use per-row calls (one
    descriptor per partition each) so compute ramps early and the
    drain overlaps the trickling tail.
  - Per supertile (4096 scores): products for the 16 ACT-reduced
    (row, col) pairs come from per-row 2x_1p tensor_tensor multiplies
    on DVE (the same multiply on Pool measures 3.4x slower — no 2x
    mode); DVE also runs 16 fused STT columns (~0.77 us each) + the
    reciprocal; ACT copy-accumulates the 16 product columns (~1.0 us
    each) + one exp; Pool issues the cast-DMAs, the within-partition
    suffix adds, and the finalize multiply. Engine loads ~18 us DVE /
    ~16.4 us ACT / ~4 us Pool against the ~20.4 us supertile DMA slot.
  - The cast-DMA issue is emitted FIRST in each loop body: Pool runs
    in order, and issuing behind dependency-stalled finalize ops
    starved the stream (measured ~8 us gap). The cross-partition sums
    T = sum_i e_i are folded into the PE matmuls by linearity (four
    accumulating matmuls per stationary, one LDWEIGHTS) instead of
    explicit adds, keeping Pool's queue short ahead of the issues.
  - Suffix-softmax state: R [128, Bc] in PSUM holds the running total
    of exp over all later supertiles, broadcast across partitions.
    Per supertile: matmuls with strictly-lower ones accumulate
    U[m] = R + sum_{p>m} sum_i e_i[p]; the divisors are the
    within-partition suffix d_i = e_i + d_{i+1}, d_3 = e_3 + U
    (4 small adds); a second set of matmuls with upper-inclusive ones
    advances R' = U + sum_{p<=m} T = R + sum T, broadcast. The finalize
    (suffix adds, reciprocal on DVE, multiply on Pool) is deferred one
    supertile so nothing waits on the exp -> matmul chain.
  - outputs collect in SBUF and DMA out per supertile on the Sync ring.
"""

import numpy as np

import concourse.bacc as bacc
import concourse.mybir as mybir
import concourse.tile as tile
from concourse import bass_utils

P = 128
S = 4096
B = 64
H = 512
N_CORES = 8
BC = B // N_CORES  # batch columns per core
R4 = 4             # seq rows per partition per supertile
SB = P * R4        # seq rows per supertile
NSUP = S // SB     # number of supertiles


def build_program(bufs16=3, look=2, acr=(5, 4, 4, 4), prod_on="dve", Bc=BC):
    """Build the per-core Bass program.

    Inputs : hs [S, Bc, H] f32,
             wb [128, H] fp16 (W broadcast across partitions),
             wbr [128, R4*act_cols*H] fp16 (W tiled for the product TT),
             trilS [128, 128] f32 ones strictly-lower (p > m),
             triuI [128, 128] f32 ones upper-inclusive (p <= m).
    Output : out [S, Bc] f32.

    act_cols: batch columns per row h-reduced by ACT copy-accumulate
    (products via one tensor_tensor on `prod_on`); the remaining
    columns are fused STT on DVE.
    """
    nsup = NSUP
    nw = R4 * Bc  # scores per partition per supertile

    nc = bacc.Bacc("TRN2", target_bir_lowering=False, debug=False)
    hs = nc.dram_tensor("hs", [S, Bc, H], mybir.dt.float32, kind="ExternalInput")
    wb = nc.dram_tensor("wb", [P, H], mybir.dt.float16, kind="ExternalInput")
    mac = max(acr)
    wbr = nc.dram_tensor(
        "wbr", [P, mac * H], mybir.dt.float16, kind="ExternalInput"
    )
    trilS = nc.dram_tensor("trilS", [P, P], mybir.dt.float32, kind="ExternalInput")
    triuI = nc.dram_tensor("triuI", [P, P], mybir.dt.float32, kind="ExternalInput")
    out = nc.dram_tensor("out", [S, Bc], mybir.dt.float32, kind="ExternalOutput")

    # partition p of supertile k holds rows t = 512k + 4p + i
    hs_r = hs.ap().rearrange("(blk p four) b h -> p blk four b h", p=P, four=R4)
    out_r = out.ap().rearrange("(blk p four) b -> p blk four b", p=P, four=R4)

    order = list(range(nsup - 1, -1, -1))  # reverse seq order
    # seq-rows-per-DMA-call by processing index: each call is one
    # contiguous descriptor per partition. Steady state is ONE call per
    # supertile — more outstanding calls overflow the SWDGE descriptor
    # ring and stall the stream. Finer calls only at the ends, where
    # compute ramp/drain overlap matters and the queue is shallow.
    # End-region call budget matters: ~4 outstanding calls are safe, 6
    # overflow the SWDGE ring (measured ~7 us late-stream stall), so only
    # the last supertile is split (per-row) for drain overlap.
    split_plan = {0: 1, 1: 2, nsup - 1: 1}
    steady_rows = R4

    with tile.TileContext(nc) as tc:
        with (
            tc.tile_pool(name="h16p", bufs=bufs16) as h16p,
            tc.tile_pool(name="consts", bufs=1) as consts,
            tc.tile_pool(name="work", bufs=1) as work,
            tc.tile_pool(name="sp", bufs=3) as sp,
            tc.tile_pool(name="ep", bufs=3) as ep,
            tc.tile_pool(name="dp", bufs=2) as dp,
            tc.tile_pool(name="prodp", bufs=2) as prodp,
            tc.tile_pool(name="psum", bufs=1, space="PSUM") as psum,
        ):
            h16_tiles = {}

            def issue_dma(idx):
                k = order[idx]
                h16 = h16p.tile([P, R4, Bc, H], mybir.dt.float16)
                nr = split_plan.get(idx, steady_rows)
                for r in range(0, R4, nr):
                    nc.gpsimd.dma_start(
                        out=h16[:, r : r + nr, :, :].rearrange(
                            "p four b h -> p (four b h)"
                        ),
                        in_=hs_r[:, k, r : r + nr, :, :].rearrange(
                            "p four b h -> p (four b h)"
                        ),
                    )
                h16_tiles[k] = h16

            wb_t = consts.tile([P, H], mybir.dt.float16)
            nc.sync.dma_start(out=wb_t, in_=wb.ap())
            wbr_t = consts.tile([P, mac, H], mybir.dt.float16)
            nc.sync.dma_start(
                out=wbr_t.rearrange("p b h -> p (b h)"), in_=wbr.ap()
            )
            trilS_t = consts.tile([P, P], mybir.dt.float32)
            nc.sync.dma_start(out=trilS_t, in_=trilS.ap())
            triuI_t = consts.tile([P, P], mybir.dt.float32)
            nc.sync.dma_start(out=triuI_t, in_=triuI.ap())

            for idx in range(look):
                issue_dma(idx)

            # Separate per-engine throwaway out-tiles: sharing one creates
            # false WAW dependencies that serialize the engines.
            dummy_v = work.tile([P, H], mybir.dt.float16)
            dummy_act = work.tile([P, H], mybir.dt.float16)
            sel_buf = work.tile([P, nsup * nw], mybir.dt.float32)
            r_ps = psum.tile([P, Bc], mybir.dt.float32)

            def emit_finalize(k, e_t, last=False):
                """Deferred: divisors, reciprocal, select, store; then R'."""
                # d_i = e_i + d_{i+1}; d_3 = e_3 + U (U is sitting in r_ps;
                # Pool cannot read PSUM, so this one add runs on DVE)
                d_t = dp.tile([P, nw], mybir.dt.float32)
                nc.vector.tensor_add(
                    d_t[:, 3 * Bc : 4 * Bc], e_t[:, 3 * Bc : 4 * Bc], r_ps
                )
                for i in (2, 1, 0):
                    nc.gpsimd.tensor_add(
                        d_t[:, i * Bc : (i + 1) * Bc],
                        e_t[:, i * Bc : (i + 1) * Bc],
                        d_t[:, (i + 1) * Bc : (i + 2) * Bc],
                    )
                rec_t = dp.tile([P, nw], mybir.dt.float32)
                nc.vector.reciprocal(rec_t, d_t)
                lo = k * nw
                nc.gpsimd.tensor_mul(sel_buf[:, lo : lo + nw], e_t, rec_t)
                nc.sync.dma_start(
                    out=out_r[:, k, :, :],
                    in_=sel_buf[:, lo : lo + nw].rearrange(
                        "p (four b) -> p four b", b=Bc
                    ),
                )
                # R' = U + sum_{p<=m} T = R + sum T, broadcast. T = sum_i e_i
                # is folded into the matmul by linearity: accumulate the four
                # e_i slices (same stationary, one LDWEIGHTS). Tile tracks
                # the WAR on r_ps (runs after the d_3 add's read of U). The
                # very last supertile's R' is never read — skip it.
                if not last:
                    for i in range(R4):
                        nc.tensor.matmul(
                            r_ps, triuI_t, e_t[:, i * Bc : (i + 1) * Bc],
                            start=False, stop=True,
                        )

            pending = None  # (k, e_t) awaiting its deferred finalize
            for idx, k in enumerate(order):
                h16 = h16_tiles[k]
                s_t = sp.tile([P, nw], mybir.dt.float32)
                e_t = ep.tile([P, nw], mybir.dt.float32)

                # Issue the next transfer FIRST: Pool executes in order, and
                # anything emitted earlier (dependency-stalled finalize adds)
                # would delay descriptor generation and starve the stream
                # (measured ~8 us mid-stream gap when issued mid-body).
                if idx + look < nsup:
                    issue_dma(idx + look)

                # Per row: a 2x_1p fp16 multiply on DVE materializes
                # products for the ACT-reduced columns (measured: the same
                # multiply on Pool runs 3.4x slower — no 2x mode there),
                # ACT copy-accumulates them, and the remaining columns run
                # as fused STT on DVE. Per-row granularity lets each row's
                # chain start right behind its DMA call during ramp/drain.
                prod_t = prodp.tile([P, R4, mac, H], mybir.dt.float16)
                is_last = idx == nsup - 1

                def emit_rows(rows):
                    for i in rows:
                        eng = nc.gpsimd if i < 2 and prod_on != "dve" else nc.vector
                        eng.tensor_tensor(
                            prod_t[:, i, : acr[i]],
                            h16[:, i, : acr[i], :],
                            wbr_t[:, : acr[i]],
                            op=mybir.AluOpType.mult,
                        )
                    for i in rows:
                        for q in range(acr[i]):
                            nc.scalar.activation(
                                dummy_act,
                                prod_t[:, i, q, :],
                                mybir.ActivationFunctionType.Copy,
                                accum_out=s_t[:, i * Bc + q : i * Bc + q + 1],
                            )
                        for b in range(acr[i], Bc):
                            nc.vector.scalar_tensor_tensor(
                                out=dummy_v,
                                in0=h16[:, i, b, :],
                                scalar=1.0,
                                in1=wb_t,
                                op0=mybir.AluOpType.mult,
                                op1=mybir.AluOpType.mult,
                                accum_out=s_t[:, i * Bc + b : i * Bc + b + 1],
                            )
                        if is_last:
                            # Per-row exp, emitted inside the row so ACT's
                            # in-order queue reaches it right after the
                            # row's reductions — shortens the drain chain
                            # behind the final DMA rows.
                            nc.scalar.activation(
                                e_t[:, i * Bc : (i + 1) * Bc],
                                s_t[:, i * Bc : (i + 1) * Bc],
                                mybir.ActivationFunctionType.Exp,
                            )

                emit_rows((0, 1))
                emit_rows((2, 3))

                # Deferred finalize of the previous supertile, emitted AFTER
                # this supertile's reductions: its d3-add (on DVE) waits on
                # the prev exp -> matmul chain, and emitting it mid-body made
                # DVE's in-order queue stall there instead of doing STTs.
                if pending is not None:
                    pk, pe = pending
                    emit_finalize(pk, pe)

                # U[m] = R + sum_{p > m} sum_i e_i[p] — T is folded into the
                # matmul by linearity (4 accumulating matmuls, one LDWEIGHTS).
                # (On the last supertile the exps were emitted per-row above;
                # each matmul fires as soon as its row's exp lands.)
                if not is_last:
                    nc.scalar.activation(
                        e_t, s_t, mybir.ActivationFunctionType.Exp
                    )
                for i in range(R4):
                    nc.tensor.matmul(
                        r_ps, trilS_t, e_t[:, i * Bc : (i + 1) * Bc],
                        start=(idx == 0 and i == 0), stop=True,
                    )
                pending = (k, e_t)

            pk, pe = pending
            emit_finalize(pk, pe, last=True)

    nc.compile()
    return nc


_PROGRAM = None


def _get_program():
    global _PROGRAM
    if _PROGRAM is None:
        _PROGRAM = build_program()
    return _PROGRAM


def make_in_maps(hidden_states, W, mac=5):
    hidden_states = np.asarray(hidden_states, dtype=np.float32)
    W = np.asarray(W, dtype=np.float32)
    wrow16 = np.broadcast_to(W[:, 0][None, :], (P, H)).astype(np.float16)
    wb = np.ascontiguousarray(wrow16)
    wbr = np.ascontiguousarray(np.tile(wrow16, (1, mac)))
    trilS = np.tril(np.ones((P, P), dtype=np.float32), -1)
    triuI = np.triu(np.ones((P, P), dtype=np.float32), 0)
    in_maps = []
    for c in range(N_CORES):
        hs_c = np.ascontiguousarray(hidden_states[:, c * BC : (c + 1) * BC, :])
        in_maps.append(
            {"hs": hs_c, "wb": wb, "wbr": wbr, "trilS": trilS, "triuI": triuI}
        )
    return in_maps


def assemble_output(results):
    cols = [results[c]["out"] for c in range(N_CORES)]
    return np.concatenate(cols, axis=1)[..., None].astype(np.float32)


def kernel(hidden_states, W, b):
    nc = _get_program()
    in_maps = make_in_maps(hidden_states, W)
    res = bass_utils.run_bass_kernel_spmd(nc, in_maps, core_ids=list(range(N_CORES)))
    return assemble_output(res.results)


# revision 47
# speedup vs baseline: 1.0306x; 1.0306x over previous
"""Trainium2 Bass kernel for suffix-softmax attention visualization.

Computes, for hidden_states [S, B, H], W [H, 1], b [1]:
    s[t, b]   = sum_h hidden_states[t, b, h] * W[h, 0] + b[0]
    out[t, b] = exp(s[t, b]) / sum_{t' >= t} exp(s[t', b])     (suffix softmax)
returned as [S, B, 1] f32.

The softmax ratio is shift-invariant, so the scalar bias b cancels exactly
and is not needed on device. Scores are N(0,1)-scaled by construction, so
exp() needs no max-subtraction.

Sharding: data-parallel over the batch axis — 8 NeuronCores, 8 batch
columns each; 64 MB of f32 per core, a pure HBM stream.

Architecture (v4, superblock). Measured context:
  - The input must land in SBUF as fp16 via the SWDGE cast-DMA: raw-f32
    landings double SBUF port traffic past the ~7 KB/us/partition wall
    (engines degrade 2.5-4x, measured 310-547 us), and only SWDGE can
    cast. HWDGE is immune to the engine-15 problem below but cannot
    cast, so it is not usable for the input.
  - Under SWDGE, SDMA engine 15 SOMETIMES (run-to-run lottery, roughly
    half of runs; device/co-tenant state) suffers periodic stall
    clusters making it ~20% slower than the other 15 engines; engines
    own fixed partition sets, so the whole stream ends up paced by
    engine 15. Separately, some runs throttle ALL engines ~17%.
  - This kernel streams in 512-seq-row SUPERBLOCKS: partition p holds
    the four consecutive rows t = 512k + 4p + i. Per partition that is
    one contiguous 64 KB f32 read -> 32 KB fp16 write descriptor — 4x
    fewer descriptors and semaphore packets than row-per-partition
    blocks and near-asymptotic per-packet efficiency (measured 163 us
    per-engine stream busy vs 169 us for the row-per-partition
    baseline; clean-run total 210.6 us vs 231.9 us).
  - Steady state issues ONE cast-DMA call per supertile: more
    outstanding calls overflow the SWDGE descriptor ring and stall the
    stream (measured 2-6 us gaps per supertile with 2 calls; same for
    look > 2). The first/last two supertiles use per-row calls (one
    descriptor per partition each) so compute ramps early and the
    drain overlaps the trickling tail.
  - Per supertile (4096 scores): products for the 16 ACT-reduced
    (row, col) pairs come from per-row 2x_1p tensor_tensor multiplies
    on DVE (the same multiply on Pool measures 3.4x slower — no 2x
    mode); DVE also runs 16 fused STT columns (~0.77 us each) + the
    reciprocal; ACT copy-accumulates the 16 product columns (~1.0 us
    each) + one exp; Pool issues the cast-DMAs, the within-partition
    suffix adds, and the finalize multiply. Engine loads ~18 us DVE /
    ~16.4 us ACT / ~4 us Pool against the ~20.4 us supertile DMA slot.
  - The cast-DMA issue is emitted FIRST in each loop body: Pool runs
    in order, and issuing behind dependency-stalled finalize ops
    starved the stream (measured ~8 us gap). The cross-partition sums
    T = sum_i e_i are folded into the PE matmuls by linearity (four
    accumulating matmuls per stationary, one LDWEIGHTS) instead of
    explicit adds, keeping Pool's queue short ahead of the issues.
  - Suffix-softmax state: R [128, Bc] in PSUM holds the running total
    of exp over all later supertiles, broadcast across partitions.
    Per supertile: matmuls with strictly-lower ones accumulate
    U[m] = R + sum_{p>m} sum_i e_i[p]; the divisors are the
    within-partition suffix d_i = e_i + d_{i+1}, d_3 = e_3 + U
    (4 small adds); a second set of matmuls with upper-inclusive ones
    advances R' = U + sum_{p<=m} T = R + sum T, broadcast. The finalize
    (suffix adds, reciprocal on DVE, multiply on Pool) is deferred one
    supertile so nothing waits on the exp -> matmul chain.
  - outputs collect in SBUF and DMA out per supertile on the Sync ring.
"""

import numpy as np

import concourse.bacc as bacc
import concourse.mybir as mybir
import concourse.tile as tile
from concourse import bass_utils

P = 128
S = 4096
B = 64
H = 512
N_CORES = 8
BC = B // N_CORES  # batch columns per core
R4 = 4             # seq rows per partition per supertile
SB = P * R4        # seq rows per supertile
NSUP = S // SB     # number of supertiles


def build_program(bufs16=3, look=2, acr=(4, 4, 4, 4), prod_on="dve", Bc=BC):
    """Build the per-core Bass program.

    Inputs : hs [S, Bc, H] f32,
             wb [128, H] fp16 (W broadcast across partitions),
             wbr [128, R4*act_cols*H] fp16 (W tiled for the product TT),
             trilS [128, 128] f32 ones strictly-lower (p > m),
             triuI [128, 128] f32 ones upper-inclusive (p <= m).
    Output : out [S, Bc] f32.

    act_cols: batch columns per row h-reduced by ACT copy-accumulate
    (products via one tensor_tensor on `prod_on`); the remaining
    columns are fused STT on DVE.
    """
    nsup = NSUP
    nw = R4 * Bc  # scores per partition per supertile

    nc = bacc.Bacc("TRN2", target_bir_lowering=False, debug=False)
    hs = nc.dram_tensor("hs", [S, Bc, H], mybir.dt.float32, kind="ExternalInput")
    wb = nc.dram_tensor("wb", [P, H], mybir.dt.float16, kind="ExternalInput")
    mac = max(acr)
    wbr = nc.dram_tensor(
        "wbr", [P, mac * H], mybir.dt.float16, kind="ExternalInput"
    )
    trilS = nc.dram_tensor("trilS", [P, P], mybir.dt.float32, kind="ExternalInput")
    triuI = nc.dram_tensor("triuI", [P, P], mybir.dt.float32, kind="ExternalInput")
    out = nc.dram_tensor("out", [S, Bc], mybir.dt.float32, kind="ExternalOutput")

    # partition p of supertile k holds rows t = 512k + 4p + i
    hs_r = hs.ap().rearrange("(blk p four) b h -> p blk four b h", p=P, four=R4)
    out_r = out.ap().rearrange("(blk p four) b -> p blk four b", p=P, four=R4)

    order = list(range(nsup - 1, -1, -1))  # reverse seq order
    # seq-rows-per-DMA-call by processing index: each call is one
    # contiguous descriptor per partition. Steady state is ONE call per
    # supertile — more outstanding calls overflow the SWDGE descriptor
    # ring and stall the stream. Finer calls only at the ends, where
    # compute ramp/drain overlap matters and the queue is shallow.
    # End-region call budget matters: ~4 outstanding calls are safe, 6
    # overflow the SWDGE ring (measured ~7 us late-stream stall), so only
    # the last supertile is split (per-row) for drain overlap.
    split_plan = {0: 1, 1: 2, nsup - 1: 1}
    steady_rows = R4

    with tile.TileContext(nc) as tc:
        with (
            tc.tile_pool(name="h16p", bufs=bufs16) as h16p,
            tc.tile_pool(name="consts", bufs=1) as consts,
            tc.tile_pool(name="work", bufs=1) as work,
            tc.tile_pool(name="sp", bufs=3) as sp,
            tc.tile_pool(name="ep", bufs=3) as ep,
            tc.tile_pool(name="dp", bufs=2) as dp,
            tc.tile_pool(name="prodp", bufs=2) as prodp,
            tc.tile_pool(name="psum", bufs=1, space="PSUM") as psum,
        ):
            h16_tiles = {}

            def issue_dma(idx):
                k = order[idx]
                h16 = h16p.tile([P, R4, Bc, H], mybir.dt.float16)
                nr = split_plan.get(idx, steady_rows)
                for r in range(0, R4, nr):
                    nc.gpsimd.dma_start(
                        out=h16[:, r : r + nr, :, :].rearrange(
                            "p four b h -> p (four b h)"
                        ),
                        in_=hs_r[:, k, r : r + nr, :, :].rearrange(
                            "p four b h -> p (four b h)"
                        ),
                    )
                h16_tiles[k] = h16

            wb_t = consts.tile([P, H], mybir.dt.float16)
            nc.sync.dma_start(out=wb_t, in_=wb.ap())
            wbr_t = consts.tile([P, mac, H], mybir.dt.float16)
            nc.sync.dma_start(
                out=wbr_t.rearrange("p b h -> p (b h)"), in_=wbr.ap()
            )
            trilS_t = consts.tile([P, P], mybir.dt.float32)
            nc.sync.dma_start(out=trilS_t, in_=trilS.ap())
            triuI_t = consts.tile([P, P], mybir.dt.float32)
            nc.sync.dma_start(out=triuI_t, in_=triuI.ap())

            for idx in range(look):
                issue_dma(idx)

            # Separate per-engine throwaway out-tiles: sharing one creates
            # false WAW dependencies that serialize the engines.
            dummy_v = work.tile([P, H], mybir.dt.float16)
            dummy_act = work.tile([P, H], mybir.dt.float16)
            sel_buf = work.tile([P, nsup * nw], mybir.dt.float32)
            r_ps = psum.tile([P, Bc], mybir.dt.float32)

            def emit_finalize(k, e_t, last=False):
                """Deferred: divisors, reciprocal, select, store; then R'."""
                # d_i = e_i + d_{i+1}; d_3 = e_3 + U (U is sitting in r_ps;
                # Pool cannot read PSUM, so this one add runs on DVE)
                d_t = dp.tile([P, nw], mybir.dt.float32)
                nc.vector.tensor_add(
                    d_t[:, 3 * Bc : 4 * Bc], e_t[:, 3 * Bc : 4 * Bc], r_ps
                )
                for i in (2, 1, 0):
                    nc.gpsimd.tensor_add(
                        d_t[:, i * Bc : (i + 1) * Bc],
                        e_t[:, i * Bc : (i + 1) * Bc],
                        d_t[:, (i + 1) * Bc : (i + 2) * Bc],
                    )
                rec_t = dp.tile([P, nw], mybir.dt.float32)
                nc.vector.reciprocal(rec_t, d_t)
                lo = k * nw
                nc.gpsimd.tensor_mul(sel_buf[:, lo : lo + nw], e_t, rec_t)
                nc.sync.dma_start(
                    out=out_r[:, k, :, :],
                    in_=sel_buf[:, lo : lo + nw].rearrange(
                        "p (four b) -> p four b", b=Bc
                    ),
                )
                # R' = U + sum_{p<=m} T = R + sum T, broadcast. T = sum_i e_i
                # is folded into the matmul by linearity: accumulate the four
                # e_i slices (same stationary, one LDWEIGHTS). Tile tracks
                # the WAR on r_ps (runs after the d_3 add's read of U). The
                # very last supertile's R' is never read — skip it.
                if not last:
                    for i in range(R4):
                        nc.tensor.matmul(
                            r_ps, triuI_t, e_t[:, i * Bc : (i + 1) * Bc],
                            start=False, stop=True,
                        )

            pending = None  # (k, e_t) awaiting its deferred finalize
            for idx, k in enumerate(order):
                h16 = h16_tiles[k]
                s_t = sp.tile([P, nw], mybir.dt.float32)
                e_t = ep.tile([P, nw], mybir.dt.float32)

                # Issue the next transfer FIRST: Pool executes in order, and
                # anything emitted earlier (dependency-stalled finalize adds)
                # would delay descriptor generation and starve the stream
                # (measured ~8 us mid-stream gap when issued mid-body).
                if idx + look < nsup:
                    issue_dma(idx + look)

                # Per row: a 2x_1p fp16 multiply on DVE materializes
                # products for the ACT-reduced columns (measured: the same
                # multiply on Pool runs 3.4x slower — no 2x mode there),
                # ACT copy-accumulates them, and the remaining columns run
                # as fused STT on DVE. Per-row granularity lets each row's
                # chain start right behind its DMA call during ramp/drain.
                prod_t = prodp.tile([P, R4, mac, H], mybir.dt.float16)
                is_last = idx == nsup - 1

                def emit_rows(rows):
                    for i in rows:
                        eng = nc.gpsimd if i < 2 and prod_on != "dve" else nc.vector
                        eng.tensor_tensor(
                            prod_t[:, i, : acr[i]],
                            h16[:, i, : acr[i], :],
                            wbr_t[:, : acr[i]],
                            op=mybir.AluOpType.mult,
                        )
                    for i in rows:
                        for q in range(acr[i]):
                            nc.scalar.activation(
                                dummy_act,
                                prod_t[:, i, q, :],
                                mybir.ActivationFunctionType.Copy,
                                accum_out=s_t[:, i * Bc + q : i * Bc + q + 1],
                            )
                        for b in range(acr[i], Bc):
                            nc.vector.scalar_tensor_tensor(
                                out=dummy_v,
                                in0=h16[:, i, b, :],
                                scalar=1.0,
                                in1=wb_t,
                                op0=mybir.AluOpType.mult,
                                op1=mybir.AluOpType.mult,
                                accum_out=s_t[:, i * Bc + b : i * Bc + b + 1],
                            )
                        if is_last:
                            # Per-row exp, emitted inside the row so ACT's
                            # in-order queue reaches it right after the
                            # row's reductions — shortens the drain chain
                            # behind the final DMA rows.
                            nc.scalar.activation(
                                e_t[:, i * Bc : (i + 1) * Bc],
                                s_t[:, i * Bc : (i + 1) * Bc],
                                mybir.ActivationFunctionType.Exp,
                            )

                emit_rows((0, 1))

                # Deferred finalize of the previous supertile: its U has been
                # sitting ready in PSUM for a while. (Emitting it after BOTH
                # row halves was measured slower — the later finalize/output
                # chain stacks into the drain.)
                if pending is not None:
                    pk, pe = pending
                    emit_finalize(pk, pe)

                emit_rows((2, 3))

                # U[m] = R + sum_{p > m} sum_i e_i[p] — T is folded into the
                # matmul by linearity (4 accumulating matmuls, one LDWEIGHTS).
                # (On the last supertile the exps were emitted per-row above;
                # each matmul fires as soon as its row's exp lands.)
                if not is_last:
                    nc.scalar.activation(
                        e_t, s_t, mybir.ActivationFunctionType.Exp
                    )
                for i in range(R4):
                    nc.tensor.matmul(
                        r_ps, trilS_t, e_t[:, i * Bc : (i + 1) * Bc],
                        start=(idx == 0 and i == 0), stop=True,
                    )
                pending = (k, e_t)

            pk, pe = pending
            emit_finalize(pk, pe, last=True)

    nc.compile()
    return nc


_PROGRAM = None


def _get_program():
    global _PROGRAM
    if _PROGRAM is None:
        _PROGRAM = build_program()
    return _PROGRAM


def make_in_maps(hidden_states, W, mac=4):
    hidden_states = np.asarray(hidden_states, dtype=np.float32)
    W = np.asarray(W, dtype=np.float32)
    wrow16 = np.broadcast_to(W[:, 0][None, :], (P, H)).astype(np.float16)
    wb = np.ascontiguousarray(wrow16)
    wbr = np.ascontiguousarray(np.tile(wrow16, (1, mac)))
    trilS = np.tril(np.ones((P, P), dtype=np.float32), -1)
    triuI = np.triu(np.ones((P, P), dtype=np.float32), 0)
    in_maps = []
    for c in range(N_CORES):
        hs_c = np.ascontiguousarray(hidden_states[:, c * BC : (c + 1) * BC, :])
        in_maps.append(
            {"hs": hs_c, "wb": wb, "wbr": wbr, "trilS": trilS, "triuI": triuI}
        )
    return in_maps


def assemble_output(results):
    cols = [results[c]["out"] for c in range(N_CORES)]
    return np.concatenate(cols, axis=1)[..., None].astype(np.float32)


def kernel(hidden_states, W, b):
    nc = _get_program()
    in_maps = make_in_maps(hidden_states, W)
    res = bass_utils.run_bass_kernel_spmd(nc, in_maps, core_ids=list(range(N_CORES)))
    return assemble_output(res.results)
